# revision 2
# baseline (speedup 1.0000x reference)
"""Background-embedding transformer layer (sparse attention) — Trainium2 Bass kernel.

Self-contained: takes FULL unsharded inputs, returns FULL output.
Shapes (hardcoded per problem spec):
  bg [8, 4, 512], feature_values [131072, 512],
  feature_batch_idx/feature_level_idx [131072] int32, max_len = 16384.

Strategy:
  - Data-parallel over batch: core b owns batch b's 16384 contiguous feature
    rows (batch_idx is sorted with equal counts).
  - RoPE rotation-transfer: <R(a)q, R(b)k> = <R(a-b)q, k>. Levels are in
    {0..3}, so the 4 rotated query variants are precomputed on host (tiny)
    and all 4 score variants are computed on device; per-key selection is
    done with one-hot per-partition masks (tensor_scalar). This removes
    RoPE over the 131072 keys entirely.
  - Flash-style fused attention: k_pad/v_pad never materialize in HBM.
    Stream 128-key tiles: load fv -> cast bf16 -> PE-transpose -> k/v GEMMs
    -> scores -> exp (no max subtraction needed; scores ~ N(0,1), a fixed
    -3 bias keeps exp small) -> accumulate attn@V and the denominator in
    persistent PSUM across all 128 tiles.
  - Post-attention block (Wo, residual, LN, FFN on [32, 512]) is tiny and
    runs on host in fp32.
"""

import sys
import numpy as np

BSZ, NLVL, E, H, FFN = 8, 4, 512, 8, 2048
D = E // H
NNZ = 131072
LMAX = NNZ // BSZ  # 16384
NT = LMAX // 128   # 128 key tiles per core
N_CORES = 8


def _layernorm(x, g, b):
    mu = x.mean(-1, keepdims=True)
    var = ((x - mu) ** 2).mean(-1, keepdims=True)
    return (x - mu) / np.sqrt(var + np.float32(1e-5)) * g + b


def _rope(x, pos, freqs):
    n = x.shape[0]
    nh, half = freqs.shape
    xr = x.reshape(n, nh, half, 2)
    ang = pos[:, None, None].astype(np.float32) * freqs[None]
    c, s = np.cos(ang), np.sin(ang)
    x1, x2 = xr[..., 0], xr[..., 1]
    out = np.stack([x1 * c - x2 * s, x1 * s + x2 * c], axis=-1)
    return out.reshape(n, -1).astype(np.float32)


# ---------------------------------------------------------------------------
# Device kernel (built/compiled once, cached)
# ---------------------------------------------------------------------------
_COMPILED = None


def _build_device_kernel():
    import concourse.bass as bass
    import concourse.tile as tile
    from concourse import bacc, mybir
    from concourse.masks import make_identity

    f32 = mybir.dt.float32
    bf16 = mybir.dt.bfloat16
    ts = bass.ts

    nc = bacc.Bacc(
        "TRN2",
        target_bir_lowering=False,
        debug=False,
        enable_asserts=False,
        num_devices=N_CORES,
    )

    fv_d = nc.dram_tensor("fv", [LMAX, E], f32, kind="ExternalInput").ap()
    wk_d = nc.dram_tensor("wk", [E, E], bf16, kind="ExternalInput").ap()
    wv_d = nc.dram_tensor("wv", [E, E], bf16, kind="ExternalInput").ap()
    q4_d = nc.dram_tensor("q4", [E, 128], bf16, kind="ExternalInput").ap()
    mask_d = nc.dram_tensor("maskl", [128, NT * 4], f32, kind="ExternalInput").ap()
    out_d = nc.dram_tensor("out", [NLVL, E], f32, kind="ExternalOutput").ap()

    with tile.TileContext(nc, trace_sim=False) as tc:
        with (
            tc.tile_pool(name="consts", bufs=1) as consts,
            tc.tile_pool(name="pf", bufs=3) as pf,
            tc.tile_pool(name="pb", bufs=2) as pb,
            tc.tile_pool(name="pt", bufs=2) as pt,
            tc.tile_pool(name="pk", bufs=2) as pk,
            tc.tile_pool(name="pv", bufs=2) as pv,
            tc.tile_pool(name="psel", bufs=2) as psel,
            tc.tile_pool(name="pexp", bufs=2) as pexp,
            tc.tile_pool(name="pfin", bufs=1) as pfin,
            tc.tile_pool(name="ppT", bufs=1, space="PSUM") as ppT,
            tc.tile_pool(name="ppk", bufs=1, space="PSUM") as ppk,
            tc.tile_pool(name="ppv", bufs=2, space="PSUM") as ppv,
            tc.tile_pool(name="pps", bufs=2, space="PSUM") as pps,
            tc.tile_pool(name="ppo", bufs=1, space="PSUM") as ppo,
        ):
            # --- one-time loads ---
            ident = consts.tile([128, 128], bf16)
            make_identity(nc, ident)
            ones_b = consts.tile([128, 1], bf16)
            nc.vector.memset(ones_b, 1.0)

            w_wk = consts.tile([128, 4, E], bf16)   # [E-chunk part, kc, hd]
            w_wv = consts.tile([128, 4, E], bf16)   # [E-chunk part, kc, hd]
            q4sb = consts.tile([128, 4, 128], bf16)  # [hd-chunk part, c, 4lvl*32]
            m_all = consts.tile([128, NT * 4], f32)  # [key part, t*4 + lvl]
            for c in range(4):
                nc.sync.dma_start(w_wk[:, c, :], wk_d[ts(c, 128), :])
                nc.sync.dma_start(w_wv[:, c, :], wv_d[ts(c, 128), :])
                nc.sync.dma_start(q4sb[:, c, :], q4_d[ts(c, 128), :])
            nc.sync.dma_start(m_all[:], mask_d[:])

            # persistent accumulators over all key tiles
            o_ps = ppo.tile([32, E], f32)       # attn @ V, rows = h*4+lq
            d_ps = ppo.tile([32, 1], f32)       # softmax denominator

            for t in range(NT):
                # load 128 feature rows [128, 512] f32
                fv_t = pf.tile([128, E], f32)
                nc.sync.dma_start(fv_t[:], fv_d[ts(t, 128), :])
                # cast to bf16
                fvb = pb.tile([128, E], bf16)
                nc.vector.tensor_copy(fvb[:], fv_t[:])
                # transpose 4x [128,128] -> fvT chunks [E-chunk part, key]
                psT = ppT.tile([128, 4, 128], f32)
                for c in range(4):
                    nc.tensor.transpose(psT[:, c], fvb[:, ts(c, 128)], ident)
                fvT = pt.tile([128, 4, 128], bf16)
                nc.scalar.copy(fvT[:], psT[:])
                # kT[hd, key]: for each hd chunk mc, accum over E chunks kc
                kT = pk.tile([128, 4, 128], bf16)
                for mc in range(4):
                    psk = ppk.tile([128, 128], f32)
                    for kc in range(4):
                        nc.tensor.matmul(
                            psk[:],
                            w_wk[:, kc, ts(mc, 128)],
                            fvT[:, kc],
                            start=(kc == 0),
                            stop=(kc == 3),
                        )
                    nc.scalar.copy(kT[:, mc], psk[:])
                # v[key, hd]: accum over E chunks
                psv = ppv.tile([128, E], f32)
                for kc in range(4):
                    nc.tensor.matmul(
                        psv[:],
                        fvT[:, kc],
                        w_wv[:, kc, :],
                        start=(kc == 0),
                        stop=(kc == 3),
                    )
                vb = pv.tile([128, E], bf16)
                nc.scalar.copy(vb[:], psv[:])
                # scores (all 4 klvl variants): [key, 4*32]
                pss = pps.tile([128, 128], f32)
                for c in range(4):
                    nc.tensor.matmul(
                        pss[:],
                        kT[:, c],
                        q4sb[:, c, :],
                        start=(c == 0),
                        stop=(c == 3),
                    )
                # select by key level: s_sel = sum_c pss[:, c*32:(c+1)*32] * m[:, t*4+c]
                ssel = psel.tile([128, 32], f32)
                stmp = psel.tile([128, 32], f32)
                nc.vector.tensor_scalar_mul(
                    ssel[:], pss[:, 0:32], m_all[:, t * 4 : t * 4 + 1]
                )
                for c in range(1, 4):
                    nc.vector.tensor_scalar_mul(
                        stmp[:],
                        pss[:, ts(c, 32)],
                        m_all[:, t * 4 + c : t * 4 + c + 1],
                    )
                    nc.vector.tensor_add(ssel[:], ssel[:], stmp[:])
                # exp (shift by -3 for range safety; cancels in the ratio)
                sexp = pexp.tile([128, 32], bf16)
                nc.scalar.activation(
                    sexp[:], ssel[:], mybir.ActivationFunctionType.Exp, bias=-3.0
                )
                # accumulate attn@V and denominator
                nc.tensor.matmul(
                    o_ps[:], sexp[:], vb[:], start=(t == 0), stop=(t == NT - 1)
                )
                nc.tensor.matmul(
                    d_ps[:], sexp[:], ones_b[:], start=(t == 0), stop=(t == NT - 1)
                )

            # finalize: out[lq, h*64+d] = o_ps[h*4+lq, h*64+d] / d_ps[h*4+lq]
            rec = pfin.tile([32, 1], f32)
            nc.vector.reciprocal(rec[:], d_ps[:])
            osc = pfin.tile([32, E], f32)
            nc.vector.tensor_scalar_mul(osc[:], o_ps[:], rec[:])
            for h in range(H):
                nc.sync.dma_start(
                    out_d[0:NLVL, ts(h, D)], osc[h * NLVL : (h + 1) * NLVL, ts(h, D)]
                )

    nc.compile()
    return nc


def _get_compiled():
    global _COMPILED
    if _COMPILED is None:
        if "/opt/trn_rl_repo" not in sys.path:
            sys.path.insert(0, "/opt/trn_rl_repo")
        _COMPILED = _build_device_kernel()
    return _COMPILED


# ---------------------------------------------------------------------------
# Host wrapper
# ---------------------------------------------------------------------------
LAST_EXEC_NS = None


def _kernel_device(bg, feature_values, feature_level_idx,
                   Wq, Wkv, Wo, rope_freqs, ln_attn_g, ln_attn_b,
                   ln_ffn_g, ln_ffn_b, W1, b1, W2, b2):
    import ml_dtypes
    from concourse import bass_utils

    bf16 = ml_dtypes.bfloat16
    nc = _get_compiled()

    # q path on host (tiny): exactly mirrors the reference quirk where
    # q rows reshape as (b, lvl) but rope positions use i // bsz.
    x = _layernorm(bg, ln_attn_g, ln_attn_b)
    q_raw = x.reshape(BSZ * NLVL, E) @ Wq  # row i = (b=i//4, lq=i%4)
    q_pos = (np.arange(BSZ * NLVL) // BSZ).astype(np.float32)  # = b//2
    scale = np.float32(1.0 / np.sqrt(D))
    q4_per_core = []
    qts = [_rope(q_raw, q_pos - np.float32(kl), rope_freqs) * scale
           for kl in range(4)]
    for b in range(BSZ):
        q4 = np.zeros((E, 128), np.float32)
        for kl in range(4):
            qt = qts[kl]
            for lq in range(NLVL):
                row = qt[b * NLVL + lq]  # [E] = (h, d)
                for h in range(H):
                    col = kl * 32 + h * 4 + lq
                    q4[h * D:(h + 1) * D, col] = row[h * D:(h + 1) * D]
        q4_per_core.append(q4.astype(bf16))

    wk_b = np.ascontiguousarray(Wkv[:, :E]).astype(bf16)
    wv_b = np.ascontiguousarray(Wkv[:, E:]).astype(bf16)

    lvl = feature_level_idx.reshape(BSZ, NT, 128)  # [b, t, p]
    in_maps = []
    for b in range(BSZ):
        onehot = (lvl[b][:, :, None] == np.arange(4, dtype=lvl.dtype)).astype(
            np.float32)                      # [t, p, 4]
        m = np.ascontiguousarray(onehot.transpose(1, 0, 2).reshape(128, NT * 4))
        in_maps.append({
            "fv": feature_values[b * LMAX:(b + 1) * LMAX],
            "wk": wk_b,
            "wv": wv_b,
            "q4": q4_per_core[b],
            "maskl": m,
        })

    res = bass_utils.run_bass_kernel_spmd(nc, in_maps, list(range(N_CORES)))
    global LAST_EXEC_NS
    LAST_EXEC_NS = res.exec_time_ns
    attn = np.stack([np.asarray(res.results[b]["out"], np.float32)
                     for b in range(BSZ)])   # [8, 4, 512]

    # post-attention block on host (tiny: [32, 512])
    o = attn @ Wo + bg
    y = _layernorm(o, ln_ffn_g, ln_ffn_b)
    y = np.maximum(y @ W1 + b1, np.float32(0.0)) @ W2 + b2
    return (o + y).astype(np.float32)


def _kernel_numpy(bg, feature_values, feature_batch_idx, feature_level_idx,
                  Wq, Wkv, Wo, rope_freqs, ln_attn_g, ln_attn_b,
                  ln_ffn_g, ln_ffn_b, W1, b1, W2, b2, max_len):
    bsz, nlvl, e = bg.shape
    nh = rope_freqs.shape[0]
    d = e // nh
    nnz = feature_values.shape[0]
    residual = bg
    x = _layernorm(bg, ln_attn_g, ln_attn_b)
    q = (x.reshape(bsz * nlvl, e) @ Wq)
    kv = feature_values @ Wkv
    k, v = kv[:, :e], kv[:, e:]
    q_lvl = (np.arange(bsz * nlvl) // bsz).astype(np.float32)
    q = _rope(q, q_lvl, rope_freqs)
    k = _rope(k, feature_level_idx.astype(np.float32), rope_freqs)
    counts = np.bincount(feature_batch_idx, minlength=bsz).astype(np.int64)
    offsets = np.concatenate([[0], np.cumsum(counts)[:-1]])
    pos = np.arange(nnz, dtype=np.int64) - offsets[feature_batch_idx]
    k_pad = np.zeros((bsz, max_len, e), np.float32)
    v_pad = np.zeros((bsz, max_len, e), np.float32)
    k_pad[feature_batch_idx, pos] = k
    v_pad[feature_batch_idx, pos] = np.ascontiguousarray(v)
    pad_mask = np.arange(max_len)[None, :] >= counts[:, None]
    qh = q.reshape(bsz, nlvl, nh, d).transpose(0, 2, 1, 3)
    kh = k_pad.reshape(bsz, max_len, nh, d).transpose(0, 2, 1, 3)
    vh = v_pad.reshape(bsz, max_len, nh, d).transpose(0, 2, 1, 3)
    scale = np.float32(1.0 / np.sqrt(np.float32(d)))
    scores = np.einsum('bhqd,bhkd->bhqk', qh, kh, optimize=True) * scale
    scores = np.where(pad_mask[:, None, None, :], np.float32(-np.inf), scores)
    m = scores.max(-1, keepdims=True)
    ex = np.exp(scores - m)
    attn = ex / ex.sum(-1, keepdims=True)
    o = np.einsum('bhqk,bhkd->bhqd', attn, vh,
                  optimize=True).transpose(0, 2, 1, 3).reshape(bsz, nlvl, e)
    o = o @ Wo + residual
    y = _layernorm(o, ln_ffn_g, ln_ffn_b)
    y = np.maximum(y @ W1 + b1, np.float32(0.0)) @ W2 + b2
    return (o + y).astype(np.float32)


def kernel(bg, feature_values, feature_batch_idx, feature_level_idx,
           Wq, Wkv, Wo, rope_freqs, ln_attn_g, ln_attn_b,
           ln_ffn_g, ln_ffn_b, W1, b1, W2, b2, max_len):
    bg = np.asarray(bg, np.float32)
    feature_values = np.ascontiguousarray(np.asarray(feature_values, np.float32))
    feature_batch_idx = np.asarray(feature_batch_idx)
    feature_level_idx = np.asarray(feature_level_idx)
    Wq = np.asarray(Wq, np.float32)
    Wkv = np.asarray(Wkv, np.float32)
    Wo = np.asarray(Wo, np.float32)
    rope_freqs = np.asarray(rope_freqs, np.float32)
    W1 = np.asarray(W1, np.float32)
    W2 = np.asarray(W2, np.float32)
    b1 = np.asarray(b1, np.float32)
    b2 = np.asarray(b2, np.float32)
    ln_attn_g = np.asarray(ln_attn_g, np.float32)
    ln_attn_b = np.asarray(ln_attn_b, np.float32)
    ln_ffn_g = np.asarray(ln_ffn_g, np.float32)
    ln_ffn_b = np.asarray(ln_ffn_b, np.float32)
    max_len = int(max_len)

    shard_ok = (
        max_len == LMAX
        and feature_values.shape == (NNZ, E)
        and np.array_equal(
            feature_batch_idx.astype(np.int64),
            np.arange(NNZ, dtype=np.int64) // LMAX,
        )
    )
    if shard_ok:
        try:
            return _kernel_device(
                bg, feature_values, feature_level_idx,
                Wq, Wkv, Wo, rope_freqs, ln_attn_g, ln_attn_b,
                ln_ffn_g, ln_ffn_b, W1, b1, W2, b2)
        except Exception:
            import traceback
            traceback.print_exc()
    return _kernel_numpy(
        bg, feature_values, feature_batch_idx, feature_level_idx,
        Wq, Wkv, Wo, rope_freqs, ln_attn_g, ln_attn_b,
        ln_ffn_g, ln_ffn_b, W1, b1, W2, b2, max_len)


# revision 5
# speedup vs baseline: 3.4354x; 3.4354x over previous
"""Background-embedding transformer layer (sparse attention) — Trainium2 Bass kernel.

Self-contained: takes FULL unsharded inputs, returns FULL output.
Shapes (hardcoded per problem spec):
  bg [8, 4, 512], feature_values [131072, 512],
  feature_batch_idx/feature_level_idx [131072] int32, max_len = 16384.

Strategy:
  - Data-parallel over batch: core b owns batch b's 16384 contiguous feature
    rows (batch_idx is sorted with equal counts).
  - RoPE rotation-transfer: <R(a)q, R(b)k> = <R(a-b)q, k>. Levels are in
    {0..3}, so the 4 rotated query variants are precomputed on host (tiny)
    and all 4 score variants are computed on device; per-key selection is
    done with one-hot per-partition masks (tensor_scalar). This removes
    RoPE over the 131072 keys entirely.
  - Flash-style fused attention: k_pad/v_pad never materialize in HBM.
    Stream 128-key tiles: load fv -> cast bf16 -> PE-transpose -> k/v GEMMs
    -> scores -> exp (no max subtraction needed; scores ~ N(0,1), a fixed
    -3 bias keeps exp small) -> accumulate attn@V and the denominator in
    persistent PSUM across all 128 tiles.
  - Post-attention block (Wo, residual, LN, FFN on [32, 512]) is tiny and
    runs on host in fp32.
"""

import sys
import numpy as np

BSZ, NLVL, E, H, FFN = 8, 4, 512, 8, 2048
D = E // H
NNZ = 131072
LMAX = NNZ // BSZ  # 16384
NT = LMAX // 128   # 128 key tiles per core
N_CORES = 8


def _layernorm(x, g, b):
    mu = x.mean(-1, keepdims=True)
    var = ((x - mu) ** 2).mean(-1, keepdims=True)
    return (x - mu) / np.sqrt(var + np.float32(1e-5)) * g + b


def _rope(x, pos, freqs):
    n = x.shape[0]
    nh, half = freqs.shape
    xr = x.reshape(n, nh, half, 2)
    ang = pos[:, None, None].astype(np.float32) * freqs[None]
    c, s = np.cos(ang), np.sin(ang)
    x1, x2 = xr[..., 0], xr[..., 1]
    out = np.stack([x1 * c - x2 * s, x1 * s + x2 * c], axis=-1)
    return out.reshape(n, -1).astype(np.float32)


# ---------------------------------------------------------------------------
# Device kernel (built/compiled once, cached)
# ---------------------------------------------------------------------------
_COMPILED = None


def _build_device_kernel():
    import concourse.bass as bass
    import concourse.tile as tile
    from concourse import bacc, mybir
    from concourse.masks import make_identity

    f32 = mybir.dt.float32
    bf16 = mybir.dt.bfloat16
    ts = bass.ts

    nc = bacc.Bacc(
        "TRN2",
        target_bir_lowering=False,
        debug=False,
        enable_asserts=False,
        num_devices=N_CORES,
    )

    fv_d = nc.dram_tensor("fv", [LMAX, E], f32, kind="ExternalInput").ap()
    wk_d = nc.dram_tensor("wk", [E, E], bf16, kind="ExternalInput").ap()
    wv_d = nc.dram_tensor("wv", [E, E], bf16, kind="ExternalInput").ap()
    q4_d = nc.dram_tensor("q4", [E, 128], bf16, kind="ExternalInput").ap()
    mask_d = nc.dram_tensor("maskl", [128, NT * 4], f32, kind="ExternalInput").ap()
    out_d = nc.dram_tensor("out", [NLVL, E], f32, kind="ExternalOutput").ap()

    with tile.TileContext(nc, trace_sim=False) as tc:
        with (
            tc.tile_pool(name="consts", bufs=1) as consts,
            tc.tile_pool(name="pf", bufs=3) as pf,
            tc.tile_pool(name="pb", bufs=2) as pb,
            tc.tile_pool(name="pt", bufs=2) as pt,
            tc.tile_pool(name="pk", bufs=2) as pk,
            tc.tile_pool(name="pv", bufs=2) as pv,
            tc.tile_pool(name="psel", bufs=2) as psel,
            tc.tile_pool(name="pexp", bufs=2) as pexp,
            tc.tile_pool(name="pfin", bufs=1) as pfin,
            tc.tile_pool(name="ppT", bufs=1, space="PSUM") as ppT,
            tc.tile_pool(name="ppk", bufs=1, space="PSUM") as ppk,
            tc.tile_pool(name="ppv", bufs=2, space="PSUM") as ppv,
            tc.tile_pool(name="pps", bufs=2, space="PSUM") as pps,
            tc.tile_pool(name="ppo", bufs=1, space="PSUM") as ppo,
        ):
            # --- one-time loads ---
            ident = consts.tile([128, 128], bf16)
            make_identity(nc, ident)
            ones_b = consts.tile([128, 1], bf16)
            nc.vector.memset(ones_b, 1.0)
            neg3 = consts.tile([128, 1], f32)
            nc.vector.memset(neg3, -3.0)

            w_wk = consts.tile([128, 4, E], bf16)   # [E-chunk part, kc, hd]
            w_wv = consts.tile([128, 4, E], bf16)   # [E-chunk part, kc, hd]
            q4sb = consts.tile([128, 4, 128], bf16)  # [hd-chunk part, c, 4lvl*32]
            m_all = consts.tile([128, NT * 4], f32)  # [key part, t*4 + lvl]
            for c in range(4):
                nc.sync.dma_start(w_wk[:, c, :], wk_d[ts(c, 128), :])
                nc.sync.dma_start(w_wv[:, c, :], wv_d[ts(c, 128), :])
                nc.sync.dma_start(q4sb[:, c, :], q4_d[ts(c, 128), :])
            nc.sync.dma_start(m_all[:], mask_d[:])

            # persistent accumulators over all key tiles
            o_ps = ppo.tile([32, E], f32)       # attn @ V, rows = h*4+lq
            d_ps = ppo.tile([32, 1], f32)       # softmax denominator

            for t in range(NT):
                # load 128 feature rows [128, 512] f32
                fv_t = pf.tile([128, E], f32)
                nc.sync.dma_start(fv_t[:], fv_d[ts(t, 128), :])
                # cast to bf16
                fvb = pb.tile([128, E], bf16)
                nc.vector.tensor_copy(fvb[:], fv_t[:])
                # transpose 4x [128,128] -> fvT chunks [E-chunk part, key]
                psT = ppT.tile([128, 4, 128], bf16)
                for c in range(4):
                    nc.tensor.transpose(psT[:, c], fvb[:, ts(c, 128)], ident)
                fvT = pt.tile([128, 4, 128], bf16)
                nc.scalar.copy(fvT[:], psT[:])
                # kT[hd, key]: for each hd chunk mc, accum over E chunks kc
                kT = pk.tile([128, 4, 128], bf16)
                for mc in range(4):
                    psk = ppk.tile([128, 128], f32)
                    for kc in range(4):
                        nc.tensor.matmul(
                            psk[:],
                            w_wk[:, kc, ts(mc, 128)],
                            fvT[:, kc],
                            start=(kc == 0),
                            stop=(kc == 3),
                        )
                    nc.scalar.copy(kT[:, mc], psk[:])
                # v[key, hd]: accum over E chunks
                psv = ppv.tile([128, E], f32)
                for kc in range(4):
                    nc.tensor.matmul(
                        psv[:],
                        fvT[:, kc],
                        w_wv[:, kc, :],
                        start=(kc == 0),
                        stop=(kc == 3),
                    )
                vb = pv.tile([128, E], bf16)
                nc.scalar.copy(vb[:], psv[:])
                # scores (all 4 klvl variants): [key, 4*32]
                pss = pps.tile([128, 128], f32)
                for c in range(4):
                    nc.tensor.matmul(
                        pss[:],
                        kT[:, c],
                        q4sb[:, c, :],
                        start=(c == 0),
                        stop=(c == 3),
                    )
                # select by key level: s_sel = sum_c pss[:, c*32:(c+1)*32] * m[:, t*4+c]
                ssel = psel.tile([128, 32], f32)
                stmp = psel.tile([128, 32], f32)
                nc.vector.tensor_scalar_mul(
                    ssel[:], pss[:, 0:32], m_all[:, t * 4 : t * 4 + 1]
                )
                for c in range(1, 4):
                    nc.vector.tensor_scalar_mul(
                        stmp[:],
                        pss[:, ts(c, 32)],
                        m_all[:, t * 4 + c : t * 4 + c + 1],
                    )
                    nc.vector.tensor_add(ssel[:], ssel[:], stmp[:])
                # exp (shift by -3 for range safety; cancels in the ratio)
                sexp = pexp.tile([128, 32], bf16)
                nc.scalar.activation(
                    sexp[:], ssel[:], mybir.ActivationFunctionType.Exp,
                    bias=neg3[:],
                )
                # accumulate attn@V and denominator
                nc.tensor.matmul(
                    o_ps[:], sexp[:], vb[:], start=(t == 0), stop=(t == NT - 1)
                )
                nc.tensor.matmul(
                    d_ps[:], sexp[:], ones_b[:], start=(t == 0), stop=(t == NT - 1)
                )

            # finalize: out[lq, h*64+d] = o_ps[h*4+lq, h*64+d] / d_ps[h*4+lq]
            rec = pfin.tile([32, 1], f32)
            nc.vector.reciprocal(rec[:], d_ps[:])
            osc = pfin.tile([32, E], f32)
            nc.vector.tensor_scalar_mul(osc[:], o_ps[:], rec[:])
            for h in range(H):
                nc.sync.dma_start(
                    out_d[0:NLVL, ts(h, D)], osc[h * NLVL : (h + 1) * NLVL, ts(h, D)]
                )

    nc.compile()
    return nc


def _get_compiled():
    global _COMPILED
    if _COMPILED is None:
        if "/opt/trn_rl_repo" not in sys.path:
            sys.path.insert(0, "/opt/trn_rl_repo")
        _COMPILED = _build_device_kernel()
    return _COMPILED


# ---------------------------------------------------------------------------
# Host wrapper
# ---------------------------------------------------------------------------
LAST_EXEC_NS = None


def _kernel_device(bg, feature_values, feature_level_idx,
                   Wq, Wkv, Wo, rope_freqs, ln_attn_g, ln_attn_b,
                   ln_ffn_g, ln_ffn_b, W1, b1, W2, b2):
    import ml_dtypes
    from concourse import bass_utils

    bf16 = ml_dtypes.bfloat16
    nc = _get_compiled()

    # q path on host (tiny): exactly mirrors the reference quirk where
    # q rows reshape as (b, lvl) but rope positions use i // bsz.
    x = _layernorm(bg, ln_attn_g, ln_attn_b)
    q_raw = x.reshape(BSZ * NLVL, E) @ Wq  # row i = (b=i//4, lq=i%4)
    q_pos = (np.arange(BSZ * NLVL) // BSZ).astype(np.float32)  # = b//2
    scale = np.float32(1.0 / np.sqrt(D))
    q4_per_core = []
    qts = [_rope(q_raw, q_pos - np.float32(kl), rope_freqs) * scale
           for kl in range(4)]
    for b in range(BSZ):
        q4 = np.zeros((E, 128), np.float32)
        for kl in range(4):
            qt = qts[kl]
            for lq in range(NLVL):
                row = qt[b * NLVL + lq]  # [E] = (h, d)
                for h in range(H):
                    col = kl * 32 + h * 4 + lq
                    q4[h * D:(h + 1) * D, col] = row[h * D:(h + 1) * D]
        q4_per_core.append(q4.astype(bf16))

    wk_b = np.ascontiguousarray(Wkv[:, :E]).astype(bf16)
    wv_b = np.ascontiguousarray(Wkv[:, E:]).astype(bf16)

    lvl = feature_level_idx.reshape(BSZ, NT, 128)  # [b, t, p]
    in_maps = []
    for b in range(BSZ):
        onehot = (lvl[b][:, :, None] == np.arange(4, dtype=lvl.dtype)).astype(
            np.float32)                      # [t, p, 4]
        m = np.ascontiguousarray(onehot.transpose(1, 0, 2).reshape(128, NT * 4))
        in_maps.append({
            "fv": feature_values[b * LMAX:(b + 1) * LMAX],
            "wk": wk_b,
            "wv": wv_b,
            "q4": q4_per_core[b],
            "maskl": m,
        })

    res = bass_utils.run_bass_kernel_spmd(nc, in_maps, list(range(N_CORES)))
    global LAST_EXEC_NS
    LAST_EXEC_NS = res.exec_time_ns
    attn = np.stack([np.asarray(res.results[b]["out"], np.float32)
                     for b in range(BSZ)])   # [8, 4, 512]

    # post-attention block on host (tiny: [32, 512])
    o = attn @ Wo + bg
    y = _layernorm(o, ln_ffn_g, ln_ffn_b)
    y = np.maximum(y @ W1 + b1, np.float32(0.0)) @ W2 + b2
    return (o + y).astype(np.float32)


def _kernel_numpy(bg, feature_values, feature_batch_idx, feature_level_idx,
                  Wq, Wkv, Wo, rope_freqs, ln_attn_g, ln_attn_b,
                  ln_ffn_g, ln_ffn_b, W1, b1, W2, b2, max_len):
    bsz, nlvl, e = bg.shape
    nh = rope_freqs.shape[0]
    d = e // nh
    nnz = feature_values.shape[0]
    residual = bg
    x = _layernorm(bg, ln_attn_g, ln_attn_b)
    q = (x.reshape(bsz * nlvl, e) @ Wq)
    kv = feature_values @ Wkv
    k, v = kv[:, :e], kv[:, e:]
    q_lvl = (np.arange(bsz * nlvl) // bsz).astype(np.float32)
    q = _rope(q, q_lvl, rope_freqs)
    k = _rope(k, feature_level_idx.astype(np.float32), rope_freqs)
    counts = np.bincount(feature_batch_idx, minlength=bsz).astype(np.int64)
    offsets = np.concatenate([[0], np.cumsum(counts)[:-1]])
    pos = np.arange(nnz, dtype=np.int64) - offsets[feature_batch_idx]
    k_pad = np.zeros((bsz, max_len, e), np.float32)
    v_pad = np.zeros((bsz, max_len, e), np.float32)
    k_pad[feature_batch_idx, pos] = k
    v_pad[feature_batch_idx, pos] = np.ascontiguousarray(v)
    pad_mask = np.arange(max_len)[None, :] >= counts[:, None]
    qh = q.reshape(bsz, nlvl, nh, d).transpose(0, 2, 1, 3)
    kh = k_pad.reshape(bsz, max_len, nh, d).transpose(0, 2, 1, 3)
    vh = v_pad.reshape(bsz, max_len, nh, d).transpose(0, 2, 1, 3)
    scale = np.float32(1.0 / np.sqrt(np.float32(d)))
    scores = np.einsum('bhqd,bhkd->bhqk', qh, kh, optimize=True) * scale
    scores = np.where(pad_mask[:, None, None, :], np.float32(-np.inf), scores)
    m = scores.max(-1, keepdims=True)
    ex = np.exp(scores - m)
    attn = ex / ex.sum(-1, keepdims=True)
    o = np.einsum('bhqk,bhkd->bhqd', attn, vh,
                  optimize=True).transpose(0, 2, 1, 3).reshape(bsz, nlvl, e)
    o = o @ Wo + residual
    y = _layernorm(o, ln_ffn_g, ln_ffn_b)
    y = np.maximum(y @ W1 + b1, np.float32(0.0)) @ W2 + b2
    return (o + y).astype(np.float32)


def kernel(bg, feature_values, feature_batch_idx, feature_level_idx,
           Wq, Wkv, Wo, rope_freqs, ln_attn_g, ln_attn_b,
           ln_ffn_g, ln_ffn_b, W1, b1, W2, b2, max_len):
    bg = np.asarray(bg, np.float32)
    feature_values = np.ascontiguousarray(np.asarray(feature_values, np.float32))
    feature_batch_idx = np.asarray(feature_batch_idx)
    feature_level_idx = np.asarray(feature_level_idx)
    Wq = np.asarray(Wq, np.float32)
    Wkv = np.asarray(Wkv, np.float32)
    Wo = np.asarray(Wo, np.float32)
    rope_freqs = np.asarray(rope_freqs, np.float32)
    W1 = np.asarray(W1, np.float32)
    W2 = np.asarray(W2, np.float32)
    b1 = np.asarray(b1, np.float32)
    b2 = np.asarray(b2, np.float32)
    ln_attn_g = np.asarray(ln_attn_g, np.float32)
    ln_attn_b = np.asarray(ln_attn_b, np.float32)
    ln_ffn_g = np.asarray(ln_ffn_g, np.float32)
    ln_ffn_b = np.asarray(ln_ffn_b, np.float32)
    max_len = int(max_len)

    shard_ok = (
        max_len == LMAX
        and feature_values.shape == (NNZ, E)
        and np.array_equal(
            feature_batch_idx.astype(np.int64),
            np.arange(NNZ, dtype=np.int64) // LMAX,
        )
    )
    if shard_ok:
        try:
            return _kernel_device(
                bg, feature_values, feature_level_idx,
                Wq, Wkv, Wo, rope_freqs, ln_attn_g, ln_attn_b,
                ln_ffn_g, ln_ffn_b, W1, b1, W2, b2)
        except Exception:
            import traceback
            traceback.print_exc()
    return _kernel_numpy(
        bg, feature_values, feature_batch_idx, feature_level_idx,
        Wq, Wkv, Wo, rope_freqs, ln_attn_g, ln_attn_b,
        ln_ffn_g, ln_ffn_b, W1, b1, W2, b2, max_len)
